# revision 1
# baseline (speedup 1.0000x reference)
"""Trainium2 Bass kernel for nn_ChunkedDynamicBlock (moe_routing).

Reference semantics (B=16384, NUM_CHUNKS=8, TOP_K=2, CHUNK=512):
    xc = x.reshape(B, 8, 512)
    activities = mean(|xc|, axis=(0, 2))                  # [8]
    topk = top_k(activities, 2)                           # descending
    ys = einsum('bki,kio->bko', xc[:, topk, :], W[topk]) + b[topk]
    out = ys.reshape(B, 1024)
    returns (out, activities, topk)

Distribution: data-parallel over batch across 8 NeuronCores (2048 rows each).
W, b replicated. Per-chunk |x| partial sums are AllReduced so every core
computes the same top-2 expert indices on-device, then each core gathers its
two selected 512-column chunks of x (dynamic-offset DMA), transposes them on
the tensor engine, and runs the two 512x512 expert matmuls.
"""

import numpy as np

import concourse.bass as bass
import concourse.tile as tile
from concourse import bacc, mybir
from concourse.bass_utils import run_bass_kernel_spmd
from concourse.masks import make_identity

N_CORES = 8
BATCH = 16384
IN_FEATURES = 4096
NUM_CHUNKS = 8
CHUNK = 512
TOP_K = 2
ROWS = BATCH // N_CORES          # 2048 rows per core
RTILES = ROWS // 128             # 16 row tiles per core
KB = CHUNK // 128                # 4 contraction sub-tiles per chunk
F32 = mybir.dt.float32
MEAN_SCALE = 1.0 / (BATCH * CHUNK)

_CACHED_NC = None


def _build():
    nc = bacc.Bacc(
        "TRN2",
        target_bir_lowering=False,
        debug=False,
        enable_asserts=False,
        num_devices=N_CORES,
    )
    x_t = nc.dram_tensor("x_shard", [ROWS, IN_FEATURES], F32, kind="ExternalInput")
    w_t = nc.dram_tensor("w_full", [NUM_CHUNKS, CHUNK, CHUNK], F32, kind="ExternalInput")
    b_t = nc.dram_tensor("b_full", [NUM_CHUNKS, CHUNK], F32, kind="ExternalInput")

    y_t = nc.dram_tensor("y_shard", [ROWS, TOP_K * CHUNK], F32, kind="ExternalOutput")
    act_t = nc.dram_tensor("act", [1, NUM_CHUNKS], F32, kind="ExternalOutput")
    topk_t = nc.dram_tensor("topk", [1, TOP_K], mybir.dt.int32, kind="ExternalOutput")

    with tile.TileContext(nc) as tc:
        with (
            tc.tile_pool(name="xin", bufs=3) as xin_pool,
            tc.tile_pool(name="stat", bufs=1) as stat_pool,
            tc.tile_pool(name="scal", bufs=1) as scal_pool,
            tc.tile_pool(name="wpool", bufs=1) as w_pool,
            tc.tile_pool(name="xsel", bufs=3) as xsel_pool,
            tc.tile_pool(name="xt", bufs=3) as xt_pool,
            tc.tile_pool(name="yout", bufs=3) as y_pool,
            tc.tile_pool(name="psx", bufs=2, space="PSUM") as psx_pool,
            tc.tile_pool(name="psy", bufs=2, space="PSUM") as psy_pool,
            tc.tile_pool(name="psb", bufs=1, space="PSUM") as psb_pool,
            tc.tile_pool(name="dram", bufs=1, space="DRAM") as dram_pool,
        ):
            # ---------------- Phase A: activities ----------------
            # acc_all[:, t*8+c] = sum_f |x[t*128+p, c*512+f]|
            acc_all = stat_pool.tile([128, RTILES * NUM_CHUNKS], F32)
            for t in range(RTILES):
                x_tile = xin_pool.tile([128, IN_FEATURES], F32)
                nc.sync.dma_start(x_tile[:], x_t.ap()[bass.ts(t, 128), :])
                nc.vector.tensor_reduce(
                    out=acc_all[:, bass.ts(t, NUM_CHUNKS)],
                    in_=x_tile[:].rearrange("p (c f) -> p c f", c=NUM_CHUNKS),
                    axis=mybir.AxisListType.X,
                    op=mybir.AluOpType.add,
                    apply_absolute_value=True,
                )
            # sum over the 16 row-tile slots -> [128, 8]
            chunk_part = stat_pool.tile([128, NUM_CHUNKS], F32)
            nc.vector.tensor_reduce(
                out=chunk_part[:],
                in_=acc_all[:].rearrange("p (t c) -> p c t", c=NUM_CHUNKS),
                axis=mybir.AxisListType.X,
                op=mybir.AluOpType.add,
            )
            # partition reduction via ones-matmul -> PSUM [1, 8]
            ones = scal_pool.tile([128, 1], F32)
            nc.vector.memset(ones[:], 1.0)
            colsum_ps = psb_pool.tile([1, NUM_CHUNKS], F32)
            nc.tensor.matmul(colsum_ps[:], ones[:], chunk_part[:], start=True, stop=True)
            colsum = scal_pool.tile([1, NUM_CHUNKS], F32)
            nc.scalar.copy(colsum[:], colsum_ps[:])

            # AllReduce the per-core chunk sums (32 bytes)
            ar_in = dram_pool.tile([1, NUM_CHUNKS], F32)
            ar_out = dram_pool.tile([1, NUM_CHUNKS], F32)
            nc.gpsimd.dma_start(ar_in[:], colsum[:])
            nc.gpsimd.collective_compute(
                "AllReduce",
                mybir.AluOpType.add,
                replica_groups=[list(range(N_CORES))],
                ins=[ar_in.opt()],
                outs=[ar_out.opt()],
            )
            act_sb = scal_pool.tile([1, NUM_CHUNKS], F32)
            nc.gpsimd.dma_start(act_sb[:], ar_out[:])
            nc.vector.tensor_scalar_mul(act_sb[:], act_sb[:], MEAN_SCALE)
            nc.sync.dma_start(act_t.ap(), act_sb[:])

            # ---------------- top-2 (descending, lowest index on ties) ------
            iota_i = scal_pool.tile([1, NUM_CHUNKS], mybir.dt.int32)
            nc.gpsimd.iota(iota_i[:], pattern=[[1, NUM_CHUNKS]], base=0,
                           channel_multiplier=0)
            iota_f = scal_pool.tile([1, NUM_CHUNKS], F32)
            nc.vector.tensor_copy(iota_f[:], iota_i[:])

            def argmax_into(src, idx_out):
                m = scal_pool.tile([1, 1], F32, tag="topk_m")
                nc.vector.tensor_reduce(out=m[:], in_=src[:],
                                        axis=mybir.AxisListType.X,
                                        op=mybir.AluOpType.max)
                eq = scal_pool.tile([1, NUM_CHUNKS], F32, tag="topk_eq")
                nc.vector.tensor_scalar(eq[:], src[:], m[0:1, 0:1], None,
                                        op0=mybir.AluOpType.is_equal)
                # candidate = iota + (1-eq)*1000
                pen = scal_pool.tile([1, NUM_CHUNKS], F32, tag="topk_pen")
                nc.vector.tensor_scalar(pen[:], eq[:], -1000.0, 1000.0,
                                        op0=mybir.AluOpType.mult,
                                        op1=mybir.AluOpType.add)
                cand = scal_pool.tile([1, NUM_CHUNKS], F32, tag="topk_cand")
                nc.vector.tensor_add(cand[:], iota_f[:], pen[:])
                nc.vector.tensor_reduce(out=idx_out, in_=cand[:],
                                        axis=mybir.AxisListType.X,
                                        op=mybir.AluOpType.min)

            idx1f = scal_pool.tile([1, 1], F32)
            argmax_into(act_sb, idx1f[:])
            # mask exactly position idx1, then find the runner-up
            mask1 = scal_pool.tile([1, NUM_CHUNKS], F32)
            nc.vector.tensor_scalar(mask1[:], iota_f[:], idx1f[0:1, 0:1], None,
                                    op0=mybir.AluOpType.is_equal)
            big = scal_pool.tile([1, NUM_CHUNKS], F32)
            nc.vector.tensor_scalar_mul(big[:], mask1[:], 1.0e30)
            act2 = scal_pool.tile([1, NUM_CHUNKS], F32)
            nc.vector.tensor_sub(act2[:], act_sb[:], big[:])
            idx2f = scal_pool.tile([1, 1], F32)
            argmax_into(act2, idx2f[:])

            topk_sb = scal_pool.tile([1, TOP_K], mybir.dt.int32)
            nc.vector.tensor_copy(topk_sb[0:1, 0:1], idx1f[:])
            nc.vector.tensor_copy(topk_sb[0:1, 1:2], idx2f[:])
            nc.sync.dma_start(topk_t.ap(), topk_sb[:])

            # ---------------- Phase B: gather + matmul ----------------
            ident = scal_pool.tile([128, 128], F32)
            make_identity(nc, ident[:])

            idx_vals = [
                nc.values_load(
                    topk_sb[0:1, r:r + 1],
                    engines=[mybir.EngineType.SP, mybir.EngineType.Pool],
                    min_val=0,
                    max_val=NUM_CHUNKS - 1,
                    skip_runtime_bounds_check=True,
                )
                for r in range(TOP_K)
            ]

            x_view = x_t.ap().rearrange("p (c f) -> p c f", c=NUM_CHUNKS)
            w_view = w_t.ap().rearrange("c (kb k) o -> c k kb o", k=128)

            for r in range(TOP_K):
                idxv = idx_vals[r]
                # expert weights: [128, kb*512] with partition = k within sub-tile
                w_sb = w_pool.tile([128, KB * CHUNK], F32, tag=f"w{r}")
                nc.sync.dma_start(
                    w_sb[:].rearrange("k (kb o) -> k kb o", kb=KB),
                    w_view[bass.ds(idxv, 1), :, :, :],
                )
                # bias: load row then broadcast to all 128 partitions
                b_row = scal_pool.tile([1, CHUNK], F32, tag=f"brow{r}")
                nc.sync.dma_start(b_row[:], b_t.ap()[bass.ds(idxv, 1), :])
                b_rep = w_pool.tile([128, CHUNK], F32, tag=f"brep{r}")
                nc.gpsimd.partition_broadcast(b_rep[:], b_row[:])

                for t in range(RTILES):
                    x_sel = xsel_pool.tile([128, CHUNK], F32)
                    nc.sync.dma_start(
                        x_sel[:],
                        x_view[bass.ts(t, 128), bass.ds(idxv, 1), :],
                    )
                    # transpose 4x [128,128] blocks into one PSUM bank
                    ps_x = psx_pool.tile([128, CHUNK], F32)
                    for fb in range(KB):
                        nc.tensor.transpose(
                            ps_x[:, bass.ts(fb, 128)],
                            x_sel[:, bass.ts(fb, 128)],
                            ident[:],
                        )
                    xt_sb = xt_pool.tile([128, CHUNK], F32)
                    nc.scalar.copy(xt_sb[:], ps_x[:])
                    # y[t] = x_chunk @ W_chunk  (accumulate over kb)
                    ps_y = psy_pool.tile([128, CHUNK], F32)
                    for fb in range(KB):
                        nc.tensor.matmul(
                            ps_y[:],
                            xt_sb[:, bass.ts(fb, 128)],
                            w_sb[:, bass.ts(fb, CHUNK)],
                            start=(fb == 0),
                            stop=(fb == KB - 1),
                        )
                    y_sb = y_pool.tile([128, CHUNK], F32)
                    nc.vector.tensor_add(y_sb[:], ps_y[:], b_rep[:])
                    nc.scalar.dma_start(
                        y_t.ap()[bass.ts(t, 128), bass.ts(r, CHUNK)], y_sb[:]
                    )

    nc.compile()
    return nc


def _get_nc():
    global _CACHED_NC
    if _CACHED_NC is None:
        _CACHED_NC = _build()
    return _CACHED_NC


def kernel(x, W, b, _trace=False, _trace_kwargs=None):
    x = np.ascontiguousarray(np.asarray(x, dtype=np.float32))
    W = np.ascontiguousarray(np.asarray(W, dtype=np.float32))
    b = np.ascontiguousarray(np.asarray(b, dtype=np.float32))
    assert x.shape == (BATCH, IN_FEATURES)
    assert W.shape == (NUM_CHUNKS, CHUNK, CHUNK)
    assert b.shape == (NUM_CHUNKS, CHUNK)

    nc = _get_nc()
    in_maps = [
        {"x_shard": x[c * ROWS:(c + 1) * ROWS], "w_full": W, "b_full": b}
        for c in range(N_CORES)
    ]
    res = run_bass_kernel_spmd(
        nc,
        in_maps,
        core_ids=list(range(N_CORES)),
        trace=_trace,
        **(_trace_kwargs or {}),
    )
    out = np.concatenate([res.results[c]["y_shard"] for c in range(N_CORES)], axis=0)
    activities = res.results[0]["act"].reshape(NUM_CHUNKS).astype(np.float32)
    topk = res.results[0]["topk"].reshape(TOP_K).astype(np.int32)
    kernel.last_results = res
    return out, activities, topk


# revision 6
# speedup vs baseline: 582.3930x; 582.3930x over previous
"""Trainium2 Bass kernel for nn_ChunkedDynamicBlock (moe_routing).

Reference semantics (B=16384, NUM_CHUNKS=8, TOP_K=2, CHUNK=512):
    xc = x.reshape(B, 8, 512)
    activities = mean(|xc|, axis=(0, 2))                  # [8]
    topk = top_k(activities, 2)                           # descending
    ys = einsum('bki,kio->bko', xc[:, topk, :], W[topk]) + b[topk]
    out = ys.reshape(B, 1024)
    returns (out, activities, topk)

Distribution: data-parallel over batch across 8 NeuronCores (2048 rows each).
W, b replicated. Per-chunk |x| partial sums are AllReduced so every core
computes the same top-2 expert indices on-device, then each core gathers its
two selected 512-column chunks of x (dynamic-offset DMA), transposes them on
the tensor engine (fp32-exact), and runs the two 512x512 expert matmuls.
"""

import numpy as np

import concourse.bass as bass
import concourse.tile as tile
from concourse import bacc, mybir
from concourse.bass_utils import run_bass_kernel_spmd
from concourse.masks import make_identity

N_CORES = 8
BATCH = 16384
IN_FEATURES = 4096
NUM_CHUNKS = 8
CHUNK = 512
TOP_K = 2
ROWS = BATCH // N_CORES          # 2048 rows per core
RTILES = ROWS // 128             # 16 row tiles per core
KB = CHUNK // 128                # 4 contraction sub-tiles per chunk
F32 = mybir.dt.float32
MEAN_SCALE = 1.0 / (BATCH * CHUNK)

_CACHED_NC = None

TUNE = {
    "xin_bufs": 3,
    "xsel_bufs": 4,
    "xt_bufs": 4,
    "yout_bufs": 4,
    "psx_bufs": 3,
    "psy_bufs": 3,
    "wb_on_gpsimd": True,
    "split_phase_a_queues": True,
    "phase_a_halves": 2,
    "stop_after": None,
}


def _build(skip_collective=False, repeat=1):
    nc = bacc.Bacc(
        "TRN2",
        target_bir_lowering=False,
        debug=False,
        enable_asserts=False,
        num_devices=1 if skip_collective else N_CORES,
    )
    x_t = nc.dram_tensor("x_shard", [ROWS, IN_FEATURES], F32, kind="ExternalInput")
    w_t = nc.dram_tensor("w_full", [NUM_CHUNKS, CHUNK, CHUNK], F32, kind="ExternalInput")
    b_t = nc.dram_tensor("b_full", [NUM_CHUNKS, CHUNK], F32, kind="ExternalInput")

    y_t = nc.dram_tensor("y_shard", [ROWS, TOP_K * CHUNK], F32, kind="ExternalOutput")
    act_t = nc.dram_tensor("act", [1, NUM_CHUNKS], F32, kind="ExternalOutput")
    topk_t = nc.dram_tensor("topk", [1, TOP_K], mybir.dt.int32, kind="ExternalOutput")

    with tile.TileContext(nc) as tc:
        with (
            tc.tile_pool(name="xin", bufs=TUNE["xin_bufs"]) as xin_pool,
            tc.tile_pool(name="stat", bufs=1) as stat_pool,
            tc.tile_pool(name="scal", bufs=1) as scal_pool,
            tc.tile_pool(name="wpool", bufs=1) as w_pool,
            tc.tile_pool(name="xsel", bufs=TUNE["xsel_bufs"]) as xsel_pool,
            tc.tile_pool(name="xt", bufs=TUNE["xt_bufs"]) as xt_pool,
            tc.tile_pool(name="yout", bufs=TUNE["yout_bufs"]) as y_pool,
            tc.tile_pool(name="psx", bufs=TUNE["psx_bufs"], space="PSUM") as psx_pool,
            tc.tile_pool(name="psy", bufs=TUNE["psy_bufs"], space="PSUM") as psy_pool,
            tc.tile_pool(name="psb", bufs=1, space="PSUM") as psb_pool,
            tc.tile_pool(name="dram", bufs=1, space="DRAM") as dram_pool,
        ):
            pools = (xin_pool, stat_pool, scal_pool, w_pool, xsel_pool,
                     xt_pool, y_pool, psx_pool, psy_pool, psb_pool, dram_pool)
            for _rep in range(repeat):
                if _rep > 0:
                    tc.strict_bb_all_engine_barrier()
                _emit_one(nc, tc, skip_collective,
                          x_t, w_t, b_t, y_t, act_t, topk_t, pools)

    nc.compile()
    return nc


def _emit_one(nc, tc, skip_collective, x_t, w_t, b_t, y_t, act_t, topk_t, pools):
    (xin_pool, stat_pool, scal_pool, w_pool, xsel_pool,
     xt_pool, y_pool, psx_pool, psy_pool, psb_pool, dram_pool) = pools

    # ---------------- Phase A: activities ----------------
    # acc_all[:, t*8 + c] = sum_f |x[t*128+p, c*512+f]|
    acc_all = stat_pool.tile([128, RTILES * NUM_CHUNKS], F32, tag="acc_all")
    NH = TUNE["phase_a_halves"]
    HALF = IN_FEATURES // NH
    HC = NUM_CHUNKS // NH
    for t in range(RTILES):
        x_tile = xin_pool.tile([128, IN_FEATURES], F32, tag="x_tile")
        eng = nc.sync if (not TUNE["split_phase_a_queues"] or t % 2 == 0) else nc.scalar
        for h in range(NH):
            eng.dma_start(
                x_tile[:, bass.ts(h, HALF)],
                x_t.ap()[bass.ts(t, 128), bass.ts(h, HALF)],
            )
            nc.vector.tensor_reduce(
                out=acc_all[:, t * NUM_CHUNKS + h * HC:
                            t * NUM_CHUNKS + (h + 1) * HC],
                in_=x_tile[:, bass.ts(h, HALF)].rearrange("p (c f) -> p c f", c=HC),
                axis=mybir.AxisListType.X,
                op=mybir.AluOpType.add,
                apply_absolute_value=True,
            )
    # sum over the 16 row-tile slots -> [128, 8]
    chunk_part = stat_pool.tile([128, NUM_CHUNKS], F32, tag="chunk_part")
    nc.vector.tensor_reduce(
        out=chunk_part[:],
        in_=acc_all[:].rearrange("p (t c) -> p c t", c=NUM_CHUNKS),
        axis=mybir.AxisListType.X,
        op=mybir.AluOpType.add,
    )
    # partition reduction via ones-matmul -> PSUM [1, 8]
    ones = scal_pool.tile([128, 1], F32, tag="ones")
    nc.vector.memset(ones[:], 1.0)
    colsum_ps = psb_pool.tile([1, NUM_CHUNKS], F32, tag="colsum_ps")
    nc.tensor.matmul(colsum_ps[:], ones[:], chunk_part[:], start=True, stop=True)
    colsum = scal_pool.tile([1, NUM_CHUNKS], F32, tag="colsum")
    nc.scalar.copy(colsum[:], colsum_ps[:])

    # AllReduce the per-core chunk sums (32 bytes)
    act_sb = scal_pool.tile([1, NUM_CHUNKS], F32, tag="act_sb")
    if skip_collective:
        # single-core modeling variant: pretend this core has it all
        nc.vector.tensor_scalar_mul(act_sb[:], colsum[:], MEAN_SCALE * N_CORES)
    else:
        ar_in = dram_pool.tile([1, NUM_CHUNKS], F32, tag="ar_in")
        ar_out = dram_pool.tile([1, NUM_CHUNKS], F32, tag="ar_out")
        nc.gpsimd.dma_start(ar_in[:], colsum[:])
        nc.gpsimd.collective_compute(
            "AllReduce",
            mybir.AluOpType.add,
            replica_groups=[list(range(N_CORES))],
            ins=[ar_in.opt()],
            outs=[ar_out.opt()],
        )
        nc.gpsimd.dma_start(act_sb[:], ar_out[:])
        nc.vector.tensor_scalar_mul(act_sb[:], act_sb[:], MEAN_SCALE)
    nc.sync.dma_start(act_t.ap(), act_sb[:])

    # ---------------- top-2 (descending, lowest index on ties) ------
    iota_i = scal_pool.tile([1, NUM_CHUNKS], mybir.dt.int32, tag="iota_i")
    nc.gpsimd.iota(iota_i[:], pattern=[[1, NUM_CHUNKS]], base=0,
                   channel_multiplier=0)
    iota_f = scal_pool.tile([1, NUM_CHUNKS], F32, tag="iota_f")
    nc.vector.tensor_copy(iota_f[:], iota_i[:])

    def argmax_into(src, idx_out, tagp):
        m = scal_pool.tile([1, 1], F32, tag=f"topk_m{tagp}")
        nc.vector.tensor_reduce(out=m[:], in_=src[:],
                                axis=mybir.AxisListType.X,
                                op=mybir.AluOpType.max)
        eq = scal_pool.tile([1, NUM_CHUNKS], F32, tag=f"topk_eq{tagp}")
        nc.vector.tensor_scalar(eq[:], src[:], m[0:1, 0:1], None,
                                op0=mybir.AluOpType.is_equal)
        # candidate = iota + (1-eq)*1000
        pen = scal_pool.tile([1, NUM_CHUNKS], F32, tag=f"topk_pen{tagp}")
        nc.vector.tensor_scalar(pen[:], eq[:], -1000.0, 1000.0,
                                op0=mybir.AluOpType.mult,
                                op1=mybir.AluOpType.add)
        cand = scal_pool.tile([1, NUM_CHUNKS], F32, tag=f"topk_cand{tagp}")
        nc.vector.tensor_add(cand[:], iota_f[:], pen[:])
        nc.vector.tensor_reduce(out=idx_out, in_=cand[:],
                                axis=mybir.AxisListType.X,
                                op=mybir.AluOpType.min)

    idx1f = scal_pool.tile([1, 1], F32, tag="idx1f")
    argmax_into(act_sb, idx1f[:], "1")
    # mask exactly position idx1, then find the runner-up
    mask1 = scal_pool.tile([1, NUM_CHUNKS], F32, tag="mask1")
    nc.vector.tensor_scalar(mask1[:], iota_f[:], idx1f[0:1, 0:1], None,
                            op0=mybir.AluOpType.is_equal)
    big = scal_pool.tile([1, NUM_CHUNKS], F32, tag="big")
    nc.vector.tensor_scalar_mul(big[:], mask1[:], 1.0e30)
    act2 = scal_pool.tile([1, NUM_CHUNKS], F32, tag="act2")
    nc.vector.tensor_sub(act2[:], act_sb[:], big[:])
    idx2f = scal_pool.tile([1, 1], F32, tag="idx2f")
    argmax_into(act2, idx2f[:], "2")

    topk_sb = scal_pool.tile([1, TOP_K], mybir.dt.int32, tag="topk_sb")
    nc.vector.tensor_copy(topk_sb[0:1, 0:1], idx1f[:])
    nc.vector.tensor_copy(topk_sb[0:1, 1:2], idx2f[:])
    nc.sync.dma_start(topk_t.ap(), topk_sb[:])

    if TUNE.get("stop_after") == "topk":
        return

    # ---------------- Phase B: gather + matmul ----------------
    ident = scal_pool.tile([128, 128], F32, tag="ident")
    make_identity(nc, ident[:])

    idx_vals = [
        nc.values_load(
            topk_sb[0:1, r:r + 1],
            engines=[mybir.EngineType.SP, mybir.EngineType.Pool],
            min_val=0,
            max_val=NUM_CHUNKS - 1,
            skip_runtime_bounds_check=True,
        )
        for r in range(TOP_K)
    ]

    x_view = x_t.ap().rearrange("p (c f) -> p c f", c=NUM_CHUNKS)
    w_view = w_t.ap().rearrange("c (kb k) o -> c k kb o", k=128)

    for r in range(TOP_K):
        idxv = idx_vals[r]
        wb_eng = nc.gpsimd if TUNE["wb_on_gpsimd"] else nc.sync
        # expert weights: [128, kb*512] with partition = k within sub-tile
        w_sb = w_pool.tile([128, KB * CHUNK], F32, tag=f"w{r}")
        wb_eng.dma_start(
            w_sb[:].rearrange("k (kb o) -> k kb o", kb=KB),
            w_view[bass.ds(idxv, 1), :, :, :],
        )
        # bias: load row then broadcast to all 128 partitions
        b_row = scal_pool.tile([1, CHUNK], F32, tag=f"brow{r}")
        wb_eng.dma_start(b_row[:], b_t.ap()[bass.ds(idxv, 1), :])
        b_rep = w_pool.tile([128, CHUNK], F32, tag=f"brep{r}")
        nc.gpsimd.partition_broadcast(b_rep[:], b_row[:])

        for t in range(RTILES):
            x_sel = xsel_pool.tile([128, CHUNK], F32, tag="x_sel")
            nc.sync.dma_start(
                x_sel[:],
                x_view[bass.ts(t, 128), bass.ds(idxv, 1), :],
            )
            # transpose 4x [128,128] blocks into one PSUM bank
            ps_x = psx_pool.tile([128, CHUNK], F32, tag="ps_x")
            for fb in range(KB):
                nc.tensor.transpose(
                    ps_x[:, bass.ts(fb, 128)],
                    x_sel[:, bass.ts(fb, 128)],
                    ident[:],
                )
            xt_sb = xt_pool.tile([128, CHUNK], F32, tag="xt_sb")
            nc.scalar.copy(xt_sb[:], ps_x[:])
            # y[t] = x_chunk @ W_chunk  (accumulate over kb)
            ps_y = psy_pool.tile([128, CHUNK], F32, tag="ps_y")
            for fb in range(KB):
                nc.tensor.matmul(
                    ps_y[:],
                    xt_sb[:, bass.ts(fb, 128)],
                    w_sb[:, bass.ts(fb, CHUNK)],
                    start=(fb == 0),
                    stop=(fb == KB - 1),
                )
            y_sb = y_pool.tile([128, CHUNK], F32, tag="y_sb")
            nc.vector.tensor_add(y_sb[:], ps_y[:], b_rep[:])
            nc.scalar.dma_start(
                y_t.ap()[bass.ts(t, 128), bass.ts(r, CHUNK)], y_sb[:]
            )


def _get_nc():
    global _CACHED_NC
    if _CACHED_NC is None:
        _CACHED_NC = _build()
    return _CACHED_NC


def kernel(x, W, b, _trace=False, _trace_kwargs=None):
    x = np.ascontiguousarray(np.asarray(x, dtype=np.float32))
    W = np.ascontiguousarray(np.asarray(W, dtype=np.float32))
    b = np.ascontiguousarray(np.asarray(b, dtype=np.float32))
    assert x.shape == (BATCH, IN_FEATURES)
    assert W.shape == (NUM_CHUNKS, CHUNK, CHUNK)
    assert b.shape == (NUM_CHUNKS, CHUNK)

    nc = _get_nc()
    in_maps = [
        {"x_shard": x[c * ROWS:(c + 1) * ROWS], "w_full": W, "b_full": b}
        for c in range(N_CORES)
    ]
    res = run_bass_kernel_spmd(
        nc,
        in_maps,
        core_ids=list(range(N_CORES)),
        trace=_trace,
        **(_trace_kwargs or {}),
    )
    out = np.concatenate([res.results[c]["y_shard"] for c in range(N_CORES)], axis=0)
    activities = res.results[0]["act"].reshape(NUM_CHUNKS).astype(np.float32)
    topk = res.results[0]["topk"].reshape(TOP_K).astype(np.int32)
    kernel.last_results = res
    return out, activities, topk
